# revision 4
# baseline (speedup 1.0000x reference)
"""Trainium2 Bass kernel for DifferentiableCassiForwardSTE (CASSI forward model).

Computation (reference):
    bands = mask2d * x[0]                               # [28, 1024, 1024]
    y[dy_l : dy_l+H, dx_l : dx_l+W] += bands[l]          # scatter-add, y: [1025, 1078]

Sharding: H (rows) across 8 cores, 128 rows each. Each core computes, for each
distinct dy value, a partial row-slab [128, Wp] holding the sum of its bands'
shifted contributions; the host scatters the slabs into the full output at row
offset dy + 128*core (the 1-row halo between cores/groups resolves via `+=`).

Per-core device kernel (exact fp32 everywhere):
  - bands stream in as 4-band / 2 MiB DMA chunks (x viewed as [p, l, c])
  - one VectorE tensor_tensor multiplies all 4 bands by the mask in place
    (mask broadcast over the band axis with a zero-stride AP)
  - dx-shift accumulation splits across two engines:
      * PE bands: plain-fp32 identity-matmul (bit-exact) accumulating into a
        per-dy-group PSUM accumulator via has_written bits
      * DVE bands: in-place tensor_tensor add into a per-group SBUF accumulator
  - tail is pipelined in column halves: ScalarE evacuates PSUM, DVE combines,
    ScalarE-ring DMA writes out.
"""

import numpy as np

L_BANDS = 28
H = 1024
W = 1024
N_CORES = 8
ROWS = H // N_CORES  # 128
P = 128
BANDS_PER_CHUNK = 4

# Bands whose dx-shift accumulation runs on the DVE (the rest go to PE),
# chosen to balance VectorE (mults + adds) against TensorE (fp32 matmuls
# at 4 cyc/row). Band 14 is the final band processed — keeping it on DVE
# shortens the exposed pipeline tail.
DVE_BANDS = frozenset({4, 9, 14, 18, 22, 26})

_cache: dict = {}


def _int_offsets(phi_d_deg, s_nom):
    # Mirrors the reference: offsets are detached host constants.
    phi = float(np.asarray(phi_d_deg).reshape(())) * np.pi / 180.0
    s = np.asarray(s_nom, dtype=np.float64)
    dx = s * np.cos(phi)
    dy = s * np.sin(phi)
    dx = dx - dx.min()
    dy = dy - dy.min()
    return np.rint(dx).astype(np.int32), np.rint(dy).astype(np.int32)


def _build(dxs: tuple, dys: tuple, dve_bands: frozenset):
    from concourse import mybir
    from concourse.bacc import Bacc
    from concourse.tile import TileContext
    from concourse.masks import make_identity

    F32 = mybir.dt.float32
    L = len(dxs)
    wp = W + max(dxs)
    group_vals = sorted(set(dys))
    n_groups = len(group_vals)
    gidx = {v: i for i, v in enumerate(group_vals)}
    band_group = [gidx[dys[l]] for l in range(L)]
    n_banks = (wp + 511) // 512
    cpb = BANDS_PER_CHUNK
    n_chunks = (L + cpb - 1) // cpb

    nc = Bacc()
    x = nc.dram_tensor("x", [L, P, W], F32, kind="ExternalInput")
    m = nc.dram_tensor("m", [P, W], F32, kind="ExternalInput")
    out = nc.dram_tensor("out", [n_groups, P, wp], F32, kind="ExternalOutput")

    with TileContext(nc) as tc:
        with (
            tc.tile_pool(name="const", bufs=1) as cpool,
            tc.tile_pool(name="raw", bufs=4) as rpool,
            tc.tile_pool(name="psum", bufs=1, space="PSUM") as ppool,
        ):
            ident = cpool.tile([P, P], F32)
            make_identity(nc, ident[:])

            mask_t = cpool.tile([P, W], F32)
            nc.sync.dma_start(out=mask_t[:], in_=m[:])

            acc_sb = [
                cpool.tile([P, wp], F32, tag=f"accsb{g}", name=f"accsb{g}")
                for g in range(n_groups)
            ]
            for g in range(n_groups):
                nc.gpsimd.memset(acc_sb[g][:], 0.0)

            acc_ps = [
                ppool.tile([P, wp], F32, space="PSUM", tag=f"accps{g}", name=f"accps{g}")
                for g in range(n_groups)
            ]
            bank_started = [[False] * n_banks for _ in range(n_groups)]
            pe_cover = [None] * n_groups  # [a, b) columns covered by PE bands

            # Process groups in reverse order (last dy group first) so every
            # group's PSUM-evac/combine/out-DMA tail except the final one
            # hides under the next group's streaming compute. Within a group,
            # band chunks shrink toward the end (4,4,...,2,1) to minimize the
            # exposed pipeline tail after the last DMA byte lands.
            def chunk_sizes(n):
                sizes = []
                while n > 3:
                    sizes.append(min(cpb, n - 3))
                    n -= sizes[-1]
                while n > 0:
                    sizes.append(min(2, n))
                    n -= sizes[-1]
                return sizes

            ci = 0
            for g in reversed(range(n_groups)):
                bands_g = [l for l in range(L) if band_group[l] == g]
                # bands within a group are contiguous l-ranges in this model
                pos = 0
                for nb in chunk_sizes(len(bands_g)):
                    ls = bands_g[pos:pos + nb]
                    pos += nb
                    l0 = ls[0]
                    assert ls == list(range(l0, l0 + nb))
                    raw = rpool.tile([P, nb, W], F32, tag=f"raw{nb}", name=f"raw{ci}")
                    ci += 1
                    nc.sync.dma_start(
                        out=raw[:], in_=x[l0:l0 + nb].rearrange("l p c -> p l c")
                    )
                    # all nb bands multiplied by the mask in one in-place
                    # pass; the mask broadcasts over the band axis
                    # (stride-0 AP dim)
                    nc.vector.tensor_tensor(
                        out=raw[:],
                        in0=raw[:],
                        in1=mask_t[:].unsqueeze(1).to_broadcast([P, nb, W]),
                        op=mybir.AluOpType.mult,
                    )
                    for j, l in enumerate(ls):
                        dx = dxs[l]
                        band = raw[:, j]
                        if l in dve_bands:
                            nc.vector.tensor_tensor(
                                out=acc_sb[g][:, dx:dx + W],
                                in0=acc_sb[g][:, dx:dx + W],
                                in1=band,
                                op=mybir.AluOpType.add,
                            )
                        else:
                            if pe_cover[g] is None:
                                pe_cover[g] = [dx, dx + W]
                            else:
                                pe_cover[g][0] = min(pe_cover[g][0], dx)
                                pe_cover[g][1] = max(pe_cover[g][1], dx + W)
                            for b in range(n_banks):
                                c0 = max(512 * b, dx)
                                c1 = min(512 * (b + 1), wp, dx + W)
                                if c0 >= c1:
                                    continue
                                nc.tensor.matmul(
                                    out=acc_ps[g][:, c0:c1],
                                    lhsT=ident[:],
                                    rhs=band[:, c0 - dx:c1 - dx],
                                    start=not bank_started[g][b],
                                    stop=False,
                                    skip_group_check=True,
                                )
                                bank_started[g][b] = True

                # Tail for this group: evacuate PSUM, combine with the DVE
                # accumulator, DMA out — pipelined in column halves.
                if pe_cover[g] is None:
                    nc.scalar.dma_start(out=out[g], in_=acc_sb[g][:])
                    continue
                a, b = pe_cover[g]
                mid_pe = (a + b) // 2
                halves = [(a, mid_pe, 0, mid_pe), (mid_pe, b, mid_pe, wp)]
                for hi, (pa, pb, oa, ob) in enumerate(halves):
                    stage = cpool.tile(
                        [P, pb - pa], F32, tag=f"stage{g}_{hi}", name=f"stage{g}_{hi}"
                    )
                    nc.scalar.copy(out=stage[:], in_=acc_ps[g][:, pa:pb])
                    nc.vector.tensor_tensor(
                        out=acc_sb[g][:, pa:pb],
                        in0=acc_sb[g][:, pa:pb],
                        in1=stage[:],
                        op=mybir.AluOpType.add,
                    )
                    nc.scalar.dma_start(
                        out=out[g][:, oa:ob], in_=acc_sb[g][:, oa:ob]
                    )

    nc.finalize()
    return nc, group_vals, wp


def _get_built(dxs, dys, dve_bands):
    key = (dxs, dys, tuple(sorted(dve_bands)))
    if key not in _cache:
        _cache[key] = _build(dxs, dys, dve_bands)
    return _cache[key]


def kernel(x_1lhw, mask2d, phi_d_deg, s_nom):
    from concourse.bass_utils import run_bass_kernel_spmd

    x_1lhw = np.asarray(x_1lhw)
    mask2d = np.asarray(mask2d)
    dx_i, dy_i = _int_offsets(phi_d_deg, s_nom)
    dxs = tuple(int(v) for v in dx_i)
    dys = tuple(int(v) for v in dy_i)
    L = x_1lhw.shape[1]
    assert x_1lhw.shape == (1, L, H, W), x_1lhw.shape
    assert len(dxs) == L

    nc, group_vals, wp = _get_built(dxs, dys, DVE_BANDS)

    in_maps = []
    for d in range(N_CORES):
        r0 = d * ROWS
        in_maps.append({
            "x": np.ascontiguousarray(x_1lhw[0, :, r0:r0 + ROWS, :], dtype=np.float32),
            "m": np.ascontiguousarray(mask2d[r0:r0 + ROWS, :], dtype=np.float32),
        })

    res = run_bass_kernel_spmd(nc, in_maps, core_ids=list(range(N_CORES)))

    hp = H + max(dys)
    y = np.zeros((hp, wp), dtype=np.float32)
    for d in range(N_CORES):
        slab = res.results[d]["out"]
        r0 = d * ROWS
        for gi, dyv in enumerate(group_vals):
            y[dyv + r0: dyv + r0 + ROWS, :] += slab[gi]

    return y[None]


# revision 21
# speedup vs baseline: 1.0904x; 1.0904x over previous
"""Trainium2 Bass kernel for DifferentiableCassiForwardSTE (CASSI forward model).

Computation (reference):
    bands = mask2d * x[0]                               # [28, 1024, 1024]
    y[dy_l : dy_l+H, dx_l : dx_l+W] += bands[l]          # scatter-add, y: [1025, 1078]

Sharding: H (rows) across 8 cores, 128 rows each. Each core computes, for each
distinct dy value, a partial row-slab [128, Wp] holding the sum of its bands'
shifted contributions; the host scatters the slabs into the full output at row
offset dy + 128*core (the 1-row halo between cores/groups resolves via `+=`).

Per-core device kernel (exact fp32 everywhere):
  - the whole x slab (14.7 MiB) streams into resident SBUF tiles in multi-band
    DMA chunks (x viewed as [p, l, c]); nothing is recycled, so the DMA stream
    issues back-to-back with no write-after-read waits
  - VectorE multiplies bands by the mask in place (mask broadcast over the
    band axis with a zero-stride AP)
  - dx-shift accumulation splits across two engines:
      * PE bands: plain-fp32 identity-matmul (bit-exact) accumulating into a
        per-dy-group PSUM accumulator via has_written bits
      * DVE bands: in-place tensor_tensor add into a per-group SBUF accumulator
  - per-group tail (pipelined in column halves): ScalarE evacuates PSUM, DVE
    combines, ScalarE-ring DMA writes out. The last dy group is processed
    first so its tail hides under the other group's streaming compute.
"""

import numpy as np

L_BANDS = 28
H = 1024
W = 1024
N_CORES = 8
ROWS = H // N_CORES  # 128
P = 128

# --- tuning knobs -----------------------------------------------------------
# Bands whose dx-shift accumulation runs on the DVE (the rest go to PE),
# chosen to balance VectorE (mults + adds) against TensorE (fp32 matmuls at
# 4 cyc/row, HAM-ramp sensitive).
DVE_BANDS = frozenset({1, 3, 6, 8, 11, 14, 16, 19, 21, 24, 26})
DMA_CHUNK = 2   # bands per input DMA
MULT_CHUNK = 1  # bands per fused in-place mask multiply
# bands whose mask multiply runs on GpSimd (keeps VectorE free for the adds)
POOL_MULT_BANDS = DVE_BANDS

_cache: dict = {}


def _int_offsets(phi_d_deg, s_nom):
    # Mirrors the reference: offsets are detached host constants.
    phi = float(np.asarray(phi_d_deg).reshape(())) * np.pi / 180.0
    s = np.asarray(s_nom, dtype=np.float64)
    dx = s * np.cos(phi)
    dy = s * np.sin(phi)
    dx = dx - dx.min()
    dy = dy - dy.min()
    return np.rint(dx).astype(np.int32), np.rint(dy).astype(np.int32)


def _split_runs(items, size, taper_tail=False, taper_head=False):
    """Split into runs of `size`; tapered ends use short (1, 2) runs so the
    pipeline starts filling / drains with minimum latency."""
    items = list(items)
    head = []
    if taper_head and len(items) > 3 and size > 1:
        head = [items[0:1], items[1:3]]
        items = items[3:]
    tail = []
    if taper_tail and len(items) > 3:
        tail = [items[-3:-1], items[-1:]]
        items = items[:-3]
    out = []
    for i in range(0, len(items), size):
        out.append(items[i:i + size])
    return head + out + tail


def _build(dxs: tuple, dys: tuple, dve_bands: frozenset,
           dma_chunk: int = DMA_CHUNK, mult_chunk: int = MULT_CHUNK,
           pool_mult_bands: frozenset = POOL_MULT_BANDS, repeat: int = 1):
    from concourse import mybir
    from concourse.bacc import Bacc
    from concourse.tile import TileContext
    from concourse.masks import make_identity

    F32 = mybir.dt.float32
    L = len(dxs)
    wp = W + max(dxs)
    group_vals = sorted(set(dys))
    n_groups = len(group_vals)
    gidx = {v: i for i, v in enumerate(group_vals)}
    band_group = [gidx[dys[l]] for l in range(L)]
    n_banks = (wp + 511) // 512

    nc = Bacc()
    x = nc.dram_tensor("x", [L, P, W], F32, kind="ExternalInput")
    m = nc.dram_tensor("m", [P, W], F32, kind="ExternalInput")
    out = nc.dram_tensor("out", [n_groups, P, wp], F32, kind="ExternalOutput")

    with TileContext(nc) as tc:
        with (
            tc.tile_pool(name="sbuf", bufs=1) as pool,
            tc.tile_pool(name="psum", bufs=1, space="PSUM") as ppool,
        ):
            ident = pool.tile([P, P], F32)
            make_identity(nc, ident[:])

            mask_t = pool.tile([P, W], F32)
            nc.sync.dma_start(out=mask_t[:], in_=m[:])

            acc_ps = [
                ppool.tile([P, wp], F32, space="PSUM", tag=f"accps{g}", name=f"accps{g}")
                for g in range(n_groups)
            ]
            group_runs = {}
            for i, g in enumerate(reversed(range(n_groups))):
                group_runs[g] = _split_runs(
                    [l for l in range(L) if band_group[l] == g],
                    dma_chunk,
                    taper_tail=(i == n_groups - 1),
                    taper_head=(i == 0),
                )

            # `repeat` re-emits the whole body (bench-only; production uses 1)
            for rep in range(repeat):
                _emit_body(
                    nc, tc, pool, mybir, rep,
                    x, out, mask_t, ident, acc_ps, group_runs,
                    dxs, band_group, dve_bands, pool_mult_bands,
                    mult_chunk, n_groups, n_banks, wp,
                )

    nc.finalize()
    return nc, group_vals, wp


def _emit_body(nc, tc, pool, mybir, rep, x, out, mask_t, ident, acc_ps,
               group_runs, dxs, band_group, dve_bands, pool_mult_bands,
               mult_chunk, n_groups, n_banks, wp):
    F32 = mybir.dt.float32
    L = len(dxs)

    if True:
        if True:
            acc_sb = [
                pool.tile([P, wp], F32, tag=f"accsb{g}", name=f"accsb{g}_{rep}")
                for g in range(n_groups)
            ]
            for g in range(n_groups):
                nc.gpsimd.memset(acc_sb[g][:], 0.0)

            bank_started = [[False] * n_banks for _ in range(n_groups)]
            pe_cover = [None] * n_groups  # [a, b) columns covered by PE bands

            # resident tiles, one per DMA chunk (contiguous l runs that do
            # not cross a group boundary, so each is one dense DRAM range);
            # the last-processed group's chunks taper (…, 2, 1) to shorten
            # the exposed pipeline tail
            chunk_tiles = {}
            for g in reversed(range(n_groups)):
                for run in group_runs[g]:
                    assert run == list(range(run[0], run[0] + len(run)))
                    l0, nb = run[0], len(run)
                    tile = pool.tile([P, nb, W], F32, tag=f"xc{l0}", name=f"xc{l0}_{rep}")
                    nc.sync.dma_start(
                        out=tile[:], in_=x[l0:l0 + nb].rearrange("l p c -> p l c")
                    )
                    for j, l in enumerate(run):
                        chunk_tiles[l] = (tile, j)

            def emit_band_accum(l):
                g = band_group[l]
                dx = dxs[l]
                tile, j = chunk_tiles[l]
                band = tile[:, j]
                if l in dve_bands:
                    nc.vector.tensor_tensor(
                        out=acc_sb[g][:, dx:dx + W],
                        in0=acc_sb[g][:, dx:dx + W],
                        in1=band,
                        op=mybir.AluOpType.add,
                    )
                    return
                if pe_cover[g] is None:
                    pe_cover[g] = [dx, dx + W]
                else:
                    pe_cover[g][0] = min(pe_cover[g][0], dx)
                    pe_cover[g][1] = max(pe_cover[g][1], dx + W)
                for b in range(n_banks):
                    c0 = max(512 * b, dx)
                    c1 = min(512 * (b + 1), wp, dx + W)
                    if c0 >= c1:
                        continue
                    nc.tensor.matmul(
                        out=acc_ps[g][:, c0:c1],
                        lhsT=ident[:],
                        rhs=band[:, c0 - dx:c1 - dx],
                        start=not bank_started[g][b],
                        stop=False,
                        skip_group_check=True,
                    )
                    bank_started[g][b] = True

            for g in reversed(range(n_groups)):
                for run in group_runs[g]:
                    # fused in-place multiplies in sub-runs of the chunk;
                    # designated bands run their multiply on GpSimd instead
                    for mrun in _split_runs(run, mult_chunk):
                        subruns = []
                        for pool_eng, grp in __import__("itertools").groupby(
                            mrun, key=lambda l: l in pool_mult_bands
                        ):
                            subruns.append((pool_eng, list(grp)))
                        for pool_eng, sr in subruns:
                            tile0, j0 = chunk_tiles[sr[0]]
                            nb = len(sr)
                            seg = tile0[:, j0:j0 + nb]
                            eng = nc.gpsimd if pool_eng else nc.vector
                            eng.tensor_tensor(
                                out=seg,
                                in0=seg,
                                in1=mask_t[:].unsqueeze(1).to_broadcast([P, nb, W]),
                                op=mybir.AluOpType.mult,
                            )
                        for l in mrun:
                            emit_band_accum(l)

                # Tail for this group: evacuate PSUM, combine with the DVE
                # accumulator, DMA out — pipelined in column halves.
                if pe_cover[g] is None:
                    nc.scalar.dma_start(out=out[g], in_=acc_sb[g][:])
                    continue
                a, b = pe_cover[g]
                mid_pe = (a + b) // 2
                halves = [(a, mid_pe, 0, mid_pe), (mid_pe, b, mid_pe, wp)]
                for hi, (pa, pb, oa, ob) in enumerate(halves):
                    stage = pool.tile(
                        [P, pb - pa], F32, tag=f"stage{g}_{hi}",
                        name=f"stage{g}_{hi}_{rep}",
                    )
                    nc.scalar.copy(out=stage[:], in_=acc_ps[g][:, pa:pb])
                    nc.vector.tensor_tensor(
                        out=acc_sb[g][:, pa:pb],
                        in0=acc_sb[g][:, pa:pb],
                        in1=stage[:],
                        op=mybir.AluOpType.add,
                    )
                    nc.scalar.dma_start(
                        out=out[g][:, oa:ob], in_=acc_sb[g][:, oa:ob]
                    )


def _get_built(dxs, dys, dve_bands, dma_chunk=DMA_CHUNK, mult_chunk=MULT_CHUNK,
               pool_mult_bands=POOL_MULT_BANDS):
    key = (dxs, dys, tuple(sorted(dve_bands)), dma_chunk, mult_chunk,
           tuple(sorted(pool_mult_bands)))
    if key not in _cache:
        _cache[key] = _build(
            dxs, dys, dve_bands, dma_chunk, mult_chunk, pool_mult_bands
        )
    return _cache[key]


def kernel(x_1lhw, mask2d, phi_d_deg, s_nom):
    from concourse.bass_utils import run_bass_kernel_spmd

    x_1lhw = np.asarray(x_1lhw)
    mask2d = np.asarray(mask2d)
    dx_i, dy_i = _int_offsets(phi_d_deg, s_nom)
    dxs = tuple(int(v) for v in dx_i)
    dys = tuple(int(v) for v in dy_i)
    L = x_1lhw.shape[1]
    assert x_1lhw.shape == (1, L, H, W), x_1lhw.shape
    assert len(dxs) == L

    nc, group_vals, wp = _get_built(dxs, dys, DVE_BANDS)

    in_maps = []
    for d in range(N_CORES):
        r0 = d * ROWS
        in_maps.append({
            "x": np.ascontiguousarray(x_1lhw[0, :, r0:r0 + ROWS, :], dtype=np.float32),
            "m": np.ascontiguousarray(mask2d[r0:r0 + ROWS, :], dtype=np.float32),
        })

    res = run_bass_kernel_spmd(nc, in_maps, core_ids=list(range(N_CORES)))

    hp = H + max(dys)
    y = np.zeros((hp, wp), dtype=np.float32)
    for d in range(N_CORES):
        slab = res.results[d]["out"]
        r0 = d * ROWS
        for gi, dyv in enumerate(group_vals):
            y[dyv + r0: dyv + r0 + ROWS, :] += slab[gi]

    return y[None]
